# revision 50
# baseline (speedup 1.0000x reference)
"""Distributed Trainium2 kernel for a pre-LN single attention block.

Reference computation (dims hardcoded):
    x: [4, 2048, 1024]; LN(x) -> q = xn@Wq, kv = xn@Wkv; 16 heads x 64;
    softmax(q k^T / 8) v ; out proj [1024,1024] + bias.

Sharding over 8 NeuronCores: core c handles batch b = c//2 and head
group g = c%2 (8 heads each).  Each core computes LN(x[b]), its
512-wide q/k/v projection slices, its 8 attention heads and a PARTIAL
out-projection; the two partials per batch are summed on the host.
gamma is folded into the projection weights on the host.

v2 design: ONE uniform 256-step pipeline (pair-major, quarter, j), no
per-quarter barriers.  Step s: scores (PE, two K=64 row-tiles co-run),
exp (ACT [128,1024]), lagged attn@v (PE, drains step s-LAG), paced
background work (projections / transposes / LN / out-proj).  The
softmax division is DEFERRED: attn@v accumulators evacuate to SBUF
unnormalized (Pool copy) while DVE takes 1/z straight from the PSUM
ones-row; the broadcast (gpsimd partition_broadcast) and the in-place
multiply run later as background items.  This keeps the quarter
boundary off every engine's critical path with psO single-buffered.

PSUM (8 banks): scores 2x[128,1024] = 4, attn@v oA/oB [65,512] = 2,
shared background pool 2x[128,512] = 2.
"""

import numpy as np
from contextlib import ExitStack

import concourse.bass as bass
import concourse.bacc as bacc_mod
import concourse.mybir as mybir
import concourse.tile as tile
from concourse.bass_utils import run_bass_kernel_spmd
from concourse.masks import make_identity

F32 = mybir.dt.float32
BF16 = mybir.dt.bfloat16
AF = mybir.ActivationFunctionType

B = 4
N = 2048          # sequence length
D = 1024          # model dim
GC = 512          # per-core inner columns (8 heads x 64)
DH = 64           # head dim
HPC = 8           # heads per core
P = 128
NT_I = N // P     # 16 sequence tiles
NT_C = D // P     # 8 model-dim tiles
NT_G = GC // P    # 4 inner tiles (= head pairs)
NQ = 4            # i-axis quarters
QW = N // NQ      # 512: quarter width
SCALE = DH ** -0.5
EPS = 1e-5
VW = HPC * (DH + 1)  # 520: v tile width incl. ones columns
LAG = 6             # attn@v drain lag (steps)
NPT = 12            # pt ring depth
NSTEP = NT_G * NQ * NT_I  # 256

MUL, ADD = mybir.AluOpType.mult, mybir.AluOpType.add

LAST_EXEC_NS = None
LAST_TRACE = None
_CACHED_NC = None


def build_nc():
    nc = bacc_mod.Bacc()
    x_d = nc.declare_dram_parameter("x", [N, D], BF16, isOutput=False)
    wq_d = nc.declare_dram_parameter("wq", [D, GC], BF16, isOutput=False)
    wk_d = nc.declare_dram_parameter("wk", [D, GC], BF16, isOutput=False)
    wv_d = nc.declare_dram_parameter("wv", [D, GC], BF16, isOutput=False)
    wo_d = nc.declare_dram_parameter("wout", [GC, D], BF16, isOutput=False)
    bo_d = nc.declare_dram_parameter("bout", [1, D], F32, isOutput=False)
    out_d = nc.declare_dram_parameter("out", [N, D], F32, isOutput=True)
    zs_d = nc.dram_tensor("zscratch", [2 * NT_G * NQ, QW], F32)

    ctx = ExitStack()
    with ctx:
        tc = ctx.enter_context(tile.TileContext(nc))

        # ---- pools live for the whole kernel -----------------------------
        const = ctx.enter_context(tc.tile_pool(name="const", bufs=1))
        wpool = ctx.enter_context(tc.tile_pool(name="wpool", bufs=1))
        small = ctx.enter_context(tc.tile_pool(name="small", bufs=4))
        ao_pool = ctx.enter_context(tc.tile_pool(name="aoT", bufs=1))
        qk_pool = ctx.enter_context(tc.tile_pool(name="qk", bufs=1))
        v_pool = ctx.enter_context(tc.tile_pool(name="vext", bufs=2))
        nrm_pool = ctx.enter_context(tc.tile_pool(name="nrm", bufs=1))
        y_pool = ctx.enter_context(tc.tile_pool(name="ybuf", bufs=3))
        xstage_cm = ctx.enter_context(tc.tile_pool(name="xstage", bufs=11))
        pt_pool = ctx.enter_context(tc.tile_pool(name="pt", bufs=1))
        xnT_pool = ctx.enter_context(tc.tile_pool(name="xnT", bufs=1,
                                                  side="right"))
        psS = ctx.enter_context(tc.tile_pool(name="psS", bufs=1,
                                             space="PSUM"))
        psO = ctx.enter_context(tc.tile_pool(name="psO", bufs=1,
                                             space="PSUM"))
        bgps = ctx.enter_context(tc.tile_pool(name="bgps", bufs=2,
                                              space="PSUM", side="right"))

        identity = const.tile([P, P], BF16, tag="identity")
        make_identity(nc, identity)
        ones_row = const.tile([1, DH], BF16, tag="ones_row")
        nc.gpsimd.memset(ones_row, 1.0)
        bout_sb = const.tile([P, D], F32, tag="bout")

        # PE p-state warm-up: junk matmuls keep PE continuously busy from
        # early in the prologue so the first real matmuls run at full clock
        jw = bgps.tile([P, 512], F32, tag="bg", name="jw")
        for _ in range(32):
            nc.tensor.matmul(jw[:, 0:P], identity, identity,
                             start=True, stop=True)

        # ---- weights: strided DMAs, issued in deadline order (wk first,
        # wo deferred into the background) so the transfers don't steal
        # DMA bandwidth from the critical early x-tile loads
        def load_w(dram, rows, cols, tagp, nsplit=1, eng=None):
            nt = rows // P
            sb = wpool.tile([P, nt * cols], BF16, tag=tagp, name=tagp)
            ts = nt // nsplit
            for sp in range(nsplit):
                (eng or nc.gpsimd).dma_start(
                    out=sb[:, sp * ts * cols:(sp + 1) * ts * cols].rearrange(
                        "p (t c) -> p t c", t=ts),
                    in_=dram[sp * ts * P:(sp + 1) * ts * P, :].rearrange(
                        "(t p) c -> p t c", p=P))
            return [sb[:, t * cols:(t + 1) * cols] for t in range(nt)]

        xnT_all = xnT_pool.tile([P, NT_C * N], BF16, tag="xnT", name="xnT")
        aoT_bf = [ao_pool.tile([P, N], BF16, tag=f"ao{t}", name=f"ao{t}")
                  for t in range(NT_G)]
        qT_bf = [qk_pool.tile([P, N], BF16, tag=f"qT{m}", name=f"qT{m}")
                 for m in range(NT_G)]
        kT_bf = [qk_pool.tile([P, N], BF16, tag=f"kT{m}", name=f"kT{m}")
                 for m in range(NT_G)]

        xn_bf = [None] * NT_I
        v_tiles = [None] * NT_I
        mvg = [None] * 4     # per group of 4 i-tiles: [P, 4, 2] (mean, var)

        # ---- LayerNorm: batched groups of 4 i-tiles (stats + batched
        # Newton rsqrt + per-tile apply).  Batching amortizes the ~0.5us
        # per-dependency-hop cost of small serial DVE chains.
        def emit_ln(i):
            g, gi = divmod(i, 4)
            if gi == 0:
                mvg[g] = small.tile([P, 4, 2], F32, tag=f"mvg{g % 2}",
                                    name=f"mvg{g}")
            xs = xstage_cm.tile([P, D], BF16, tag="xst")
            nc.sync.dma_start(out=xs, in_=x_d[i * P:(i + 1) * P, :])
            stats = small.tile([P, 2, 6], F32, tag="stats")
            for sg in range(2):
                nc.vector.bn_stats(out=stats[:, sg, :],
                                   in_=xs[:, sg * 512:(sg + 1) * 512])
            nc.vector.bn_aggr(out=mvg[g][:, gi, :], in_=stats)
            xn_bf[i] = xs

        def emit_ln_group(g):
            # The whole chain runs at high priority: its ~8 serial hops
            # otherwise interleave with queued stats work, costing ~0.7us
            # per hop of added latency on the critical path to the
            # transposes.
            mv = mvg[g]
            with tc.high_priority():
                veps = small.tile([P, 4], F32, tag=f"veps{g % 2}")
                nc.vector.tensor_scalar(veps, mv[:, :, 1], EPS, None,
                                        op0=ADD)
                y = small.tile([P, 4], F32, tag=f"nry{g % 2}")
                nc.vector.tensor_scalar(y, veps, -0.5, 1.5, op0=MUL, op1=ADD)
                for _ in range(2):
                    a = small.tile([P, 4], F32, tag="nra")
                    nc.vector.tensor_mul(a, y, y)
                    nc.vector.scalar_tensor_tensor(a, a, -0.5, veps,
                                                   op0=MUL, op1=MUL)
                    # y <- (a + 1.5) * y in one fused hop
                    nc.vector.scalar_tensor_tensor(y, a, 1.5, y,
                                                   op0=ADD, op1=MUL)
                for gi in range(4):
                    i = 4 * g + gi
                    nb = small.tile([P, 1], F32, tag="nb")
                    nc.vector.scalar_tensor_tensor(nb, mv[:, gi, 0:1], -1.0,
                                                   y[:, gi:gi + 1],
                                                   op0=MUL, op1=MUL)
                    nc.vector.tensor_scalar(xn_bf[i], xn_bf[i],
                                            y[:, gi:gi + 1], nb,
                                            op0=MUL, op1=ADD)

        def emit_tr(i):
            # transpose xn[i] -> xnT columns, via identity matmuls,
            # in two 4-ct chunks through the shared background pool
            for half in range(2):
                ps = bgps.tile([P, 512], F32, tag="bg")
                for c4 in range(4):
                    ct = half * 4 + c4
                    nc.tensor.matmul(ps[:, c4 * P:(c4 + 1) * P],
                                     xn_bf[i][:, ct * P:(ct + 1) * P],
                                     identity, start=True, stop=True)
                nc.vector.tensor_copy(
                    out=xnT_all[:, :].rearrange("p (ct i) -> p ct i", ct=NT_C)[:, half * 4:half * 4 + 4, i * P:(i + 1) * P],
                    in_=ps[:].rearrange("p (c4 i) -> p c4 i", i=P))

        def emit_v(i, half):
            # project 4 heads (pairs 2*half, 2*half+1) of v for i-tile i.
            # The low half is needed from pair 0; the high half only from
            # pair 2, so it runs as late background work.
            if half == 0:
                vt = v_pool.tile([P, VW], BF16, tag=f"v{i}", name=f"v{i}",
                                 bufs=1)
                nc.gpsimd.memset(vt, 1.0)
                v_tiles[i] = vt
            else:
                vt = v_tiles[i]
            psv = bgps.tile([P, 512], F32, tag="bg")
            for ct in range(NT_C):
                nc.tensor.matmul(psv[:, 0:256],
                                 xnT_all[:, ct * N + i * P:ct * N + (i + 1) * P],
                                 wv_bf[ct][:, half * 256:(half + 1) * 256],
                                 start=(ct == 0), stop=(ct == NT_C - 1))
            nc.vector.tensor_copy(
                out=vt[:, half * VW // 2:(half + 1) * VW // 2].rearrange(
                    "p (h e) -> p h e", h=HPC // 2)[:, :, 0:DH],
                in_=psv[:, 0:256].rearrange("p (h e) -> p h e", e=DH))

        def proj_chunk(w_bf, ot, m, nck, width=512):
            ps = bgps.tile([P, 512], F32, tag="bg")
            c0 = nck * width
            for ct in range(NT_C):
                nc.tensor.matmul(ps[:, 0:width],
                                 w_bf[ct][:, m * P:(m + 1) * P],
                                 xnT_all[:, ct * N + c0:ct * N + c0 + width],
                                 start=(ct == 0), stop=(ct == NT_C - 1))
            nc.vector.tensor_copy(out=ot[:, c0:c0 + width], in_=ps[:, 0:width])

        def emit_outproj_half(q, it, nck, psy=None, t0=0):
            i0 = q * QW + it * P
            ys = y_pool.tile([P, 512], F32, tag="ys")
            cont = psy is not None
            if psy is None:
                psy = bgps.tile([P, 512], F32, tag="bg")
            for t in range(t0, NT_G):
                nc.tensor.matmul(psy,
                                 aoT_bf[t][:, i0:i0 + P],
                                 wo_bf[t][:, nck * 512:(nck + 1) * 512],
                                 start=(t == t0 and not cont),
                                 stop=(t == NT_G - 1),
                                 skip_group_check=cont)
            nc.vector.tensor_add(ys, psy,
                                 bout_sb[:, nck * 512:(nck + 1) * 512])
            nc.sync.dma_start(out=out_d[i0:i0 + P, nck * 512:(nck + 1) * 512],
                              in_=ys)

        def emit_outproj_pre(q, it, nck):
            # accumulate pairs 0-2 (already normalized) into a held psum
            # bank; the pair-3 contribution lands in emit_outproj_half.
            # Used only in the tail where the bg pool is otherwise idle.
            i0 = q * QW + it * P
            psy = bgps.tile([P, 512], F32, tag="bg")
            for t in range(3):
                nc.tensor.matmul(psy,
                                 aoT_bf[t][:, i0:i0 + P],
                                 wo_bf[t][:, nck * 512:(nck + 1) * 512],
                                 start=(t == 0), stop=False,
                                 skip_group_check=True)
            return psy

        # ---- background queues -------------------------------------------
        # static items: (due_step, prefetch_horizon, heavy, fn); an item may
        # be emitted from step (due - horizon) onward, and is forced at its
        # due step.  At most one "heavy" (PE-costly) item per step unless
        # forced.  Dynamic items (deferred normalize, out-proj) are appended
        # at runtime and drained one per step.
        bg_items = []
        dyn_items = []

        def BG(due, hor, heavy, fn):
            bg_items.append([due, hor, heavy, fn])

        def LN(i):
            return lambda: emit_ln(i)

        def LNG(g):
            return lambda: emit_ln_group(g)

        def TR(i):
            return lambda: emit_tr(i)

        def V(i, half):
            return lambda: emit_v(i, half)

        def K(m, c, w=512):
            return lambda: proj_chunk(wk_bf, kT_bf[m], m, c, w)

        def Q(m, c):
            return lambda: proj_chunk(wq_bf, qT_bf[m], m, c)

        def OP(q, it, nck):
            return lambda: emit_outproj_half(q, it, nck)

        def WO():
            def fn():
                wo_bf.extend(load_w(wo_d, GC, D, "wo"))
                nc.gpsimd.dma_start(out=bout_sb,
                                    in_=bo_d[0:1, :].to_broadcast((P, D)))
            return fn

        for g in range(1, 4):
            for gi in range(4):
                BG(4 * g - 4, 12, False, LN(4 * g + gi))
            BG(4 * g - 3, 8, False, LNG(g))
        for i in range(4, NT_I):
            BG(i - 2, 8, True, TR(i))
        for c in range(2, 8):             # pair-0 k: 256-wide chunks
            BG(2 * c - 1, 6, True, K(0, c, 256))
        for i in range(NT_I):
            BG(i + LAG - 1, 3, True, V(i, 0))
            BG(75 + i, 10, True, V(i, 1))
        for c in range(1, 4):
            BG(16 * c - 2, 6, True, Q(0, c))
        BG(40, 0, False, WO())
        for p in range(1, NT_G):
            for c in range(4):
                BG(64 * p + 4 * c - 6, 10, True, K(p, c))
                BG(64 * p + 16 * c - 6, 6, True, Q(p, c))
        bg_items.sort(key=lambda x: x[0])

        # ---- steady-state emitters ---------------------------------------
        pt_ring = [None] * NPT
        oAB = [None, None]

        def emit_scores_exp(s):
            p, q, j = s // 64, (s // 16) % 4, s % 16
            kt, qt = kT_bf[p], qT_bf[p]
            ps = psS.tile([P, 2 * QW], F32, tag=f"s{s % 2}", name=f"ps{s}")
            nc.tensor.matmul(ps[:, 0:QW],
                             kt[0:DH, j * P:(j + 1) * P],
                             qt[0:DH, q * QW:(q + 1) * QW],
                             start=True, stop=True)
            nc.tensor.matmul(ps[:, QW:2 * QW],
                             kt[DH:P, j * P:(j + 1) * P],
                             qt[DH:P, q * QW:(q + 1) * QW],
                             start=True, stop=True)
            pt = pt_pool.tile([P, 2 * QW], BF16, tag=f"pt{s % NPT}",
                              name=f"pt{s}")
            nc.scalar.activation(out=pt, in_=ps, func=AF.Exp, scale=SCALE)
            pt_ring[s % NPT] = pt

        def norm_bg(p, q, rzbf):
            # deferred: broadcast 1/z along partitions, then scale both
            # heads' aoT halves with one in-place multiply.  Pairs 0-2 use
            # a DRAM round-trip (latency fully hidden — this runs many
            # steps later); pair 3 broadcasts via K=1 PE matmuls (bf16 1/z)
            # to keep the crowded final region off the DMA queues.
            slot = (p * NQ + q) * 2
            def fn():
                if rzbf is None:
                    rb = nrm_pool.tile([P, QW], F32, tag="rb", bufs=2,
                                       name="rb")
                    for h in range(2):
                        nc.sync.dma_start(
                            out=rb[h * DH:(h + 1) * DH, :],
                            in_=zs_d[slot + h:slot + h + 1, :].to_broadcast(
                                (DH, QW)))
                else:
                    rb = bgps.tile([P, 512], F32, tag="bg", name="rbps")
                    for h in range(2):
                        nc.tensor.matmul(rb[h * DH:(h + 1) * DH, :],
                                         ones_row, rzbf[h],
                                         start=True, stop=True)
                sl = aoT_bf[p][:, q * QW:(q + 1) * QW]
                nc.vector.tensor_mul(sl, sl, rb)
            return fn

        def emit_attnv(t):
            p, q, j = t // 64, (t // 16) % 4, t % 16
            vt = v_tiles[j]
            ptt = pt_ring[t % NPT]
            if j == 0:
                oAB[0] = psO.tile([DH + 1, QW], F32, tag="oa", name=f"oA{t}")
                oAB[1] = psO.tile([DH + 1, QW], F32, tag="ob", name=f"oB{t}")
            v0 = p * 2 * (DH + 1)
            for h in range(2):
                nc.tensor.matmul(oAB[h],
                                 vt[:, v0 + h * (DH + 1):v0 + (h + 1) * (DH + 1)],
                                 ptt[:, h * QW:(h + 1) * QW],
                                 start=(j == 0), stop=(j == NT_I - 1),
                                 skip_group_check=True)
            if j == NT_I - 1:
                last = p == NT_G - 1
                rzbf = [None, None] if last else None
                with tc.high_priority():
                    for h in range(2):
                        o_ps = oAB[h]
                        zr = nrm_pool.tile([1, QW], F32, tag="zr", bufs=2,
                                           name=f"zr{t}_{h}")
                        nc.vector.tensor_copy(out=zr, in_=o_ps[DH:DH + 1, :])
                        rz = nrm_pool.tile([1, QW], F32, tag=f"rz{h}",
                                           bufs=2, name=f"rz{t}_{h}")
                        nc.vector.reciprocal_approx_fast(out=rz, in_=zr)
                        nc.vector.tensor_copy(
                            out=aoT_bf[p][h * DH:(h + 1) * DH,
                                          q * QW:(q + 1) * QW],
                            in_=o_ps[0:DH, :])
                        if last:
                            rzb = nrm_pool.tile([1, QW], BF16,
                                                tag=f"rzb{h}", bufs=2,
                                                name=f"rzb{t}_{h}")
                            nc.vector.tensor_copy(out=rzb, in_=rz)
                            rzbf[h] = rzb
                        else:
                            slot = (p * NQ + q) * 2 + h
                            nc.sync.dma_start(out=zs_d[slot:slot + 1, :],
                                              in_=rz)
                dyn_items.append(norm_bg(p, q, rzbf))
                if last and q < NQ - 1:
                    # out-projection tiles of this quarter become available
                    # once the deferred multiplies above run
                    for it in range(4):
                        for nck in range(2):
                            dyn_items.append(OP(q, it, nck))

        # ---- prologue ----------------------------------------------------
        # x tiles 0-3 FIRST on the DMA queues, then weights in deadline
        # order: the weight transfers must not delay the x loads that gate
        # LayerNorm.
        for i in range(4):
            emit_ln(i)
        # weights ride the SAME sync queue as the x tiles, behind x0-3:
        # in-queue order guarantees the critical x loads transfer first
        wk_bf = load_w(wk_d, D, GC, "wk", nsplit=2, eng=nc.sync)
        wq_bf = load_w(wq_d, D, GC, "wq", nsplit=2, eng=nc.sync)
        wv_bf = load_w(wv_d, D, GC, "wv", nsplit=2, eng=nc.sync)
        wo_bf = []
        emit_ln_group(0)
        for i in range(4):
            emit_tr(i)
        for c in range(2):
            proj_chunk(wk_bf, kT_bf[0], 0, c, 256)
        proj_chunk(wq_bf, qT_bf[0], 0, 0)

        # ---- main pipeline -----------------------------------------------
        bg_pos = 0

        def drain_bg(s):
            nonlocal bg_pos
            heavy_done = 0
            light_done = 0
            while bg_pos < len(bg_items):
                due, hor, heavy, fn = bg_items[bg_pos]
                if due <= s:
                    pass  # forced
                elif due - hor <= s and (
                        (heavy and heavy_done < 1)
                        or (not heavy and light_done < 2)):
                    pass
                else:
                    break
                fn()
                bg_pos += 1
                if heavy:
                    heavy_done += 1
                else:
                    light_done += 1
            if dyn_items:
                dyn_items.pop(0)()

        next_av = 0
        for s in range(NSTEP):
            emit_scores_exp(s)
            if s >= LAG:
                emit_attnv(next_av)
                next_av += 1
                if s >= NSTEP - LAG:
                    # tail: ramp the drain lag down so the last attn@v
                    # work lands right behind the last exps
                    emit_attnv(next_av)
                    next_av += 1
            drain_bg(s)
        assert next_av == NSTEP
        while bg_pos < len(bg_items):
            bg_items[bg_pos][3]()
            bg_pos += 1
        while dyn_items:
            dyn_items.pop(0)()
        # final quarter's out-projection: the pairs-0-2 partial sums are
        # ready now and warm the PE while the last normalize chain runs;
        # pair 3's contribution + bias + store follow.  Two held psum
        # banks pipeline the eight half-tiles.
        halves = [(it, nck) for it in range(4) for nck in range(2)]
        pend = []
        for idx, (it, nck) in enumerate(halves):
            pend.append((it, nck, emit_outproj_pre(3, it, nck)))
            if len(pend) == 2 or idx == len(halves) - 1:
                while pend:
                    pit, pnck, ppsy = pend.pop(0)
                    emit_outproj_half(3, pit, pnck, psy=ppsy, t0=3)

    nc.compile()
    return nc


def kernel(x, gamma, Wq, Wkv, Wout, bout, _trace=False, _tmpdir=None):
    global _CACHED_NC, LAST_EXEC_NS, LAST_TRACE
    x = np.asarray(x, dtype=np.float32)
    gamma = np.asarray(gamma, dtype=np.float32)
    Wq = np.asarray(Wq, dtype=np.float32)
    Wkv = np.asarray(Wkv, dtype=np.float32)
    Wout = np.asarray(Wout, dtype=np.float32)
    bout = np.asarray(bout, dtype=np.float32)

    # fold LN gamma into the projection weights (exact), cast to bf16
    import ml_dtypes
    bf = ml_dtypes.bfloat16
    Wqg = (gamma[:, None] * Wq).astype(bf)
    Wk = (gamma[:, None] * Wkv[:, :D]).astype(bf)
    Wv = (gamma[:, None] * Wkv[:, D:]).astype(bf)
    Wo_b = Wout.astype(bf)
    x_b = x.astype(bf)
    zeros_b = np.zeros((1, D), dtype=np.float32)

    in_maps = []
    for c in range(8):
        b, g = divmod(c, 2)
        sl = slice(g * GC, (g + 1) * GC)
        in_maps.append({
            "x": np.ascontiguousarray(x_b[b]),
            "wq": np.ascontiguousarray(Wqg[:, sl]),
            "wk": np.ascontiguousarray(Wk[:, sl]),
            "wv": np.ascontiguousarray(Wv[:, sl]),
            "wout": np.ascontiguousarray(Wo_b[sl, :]),
            "bout": bout.reshape(1, D) if g == 0 else zeros_b,
        })

    if _CACHED_NC is None:
        _CACHED_NC = build_nc()
    nc = _CACHED_NC

    kw = {}
    if _trace:
        import concourse.bass_utils as bu
        bu.upload_artifacts = lambda tmpdir: "not-uploaded"
        kw = dict(trace=True, tmpdir=_tmpdir)
    try:
        res = run_bass_kernel_spmd(nc, in_maps, core_ids=list(range(8)), **kw)
    except Exception:
        # transient device faults (e.g. NRT_EXEC_UNIT_UNRECOVERABLE) clear on
        # a fresh attempt; retry once before giving up
        res = run_bass_kernel_spmd(nc, in_maps, core_ids=list(range(8)), **kw)
    LAST_EXEC_NS = res.exec_time_ns
    LAST_TRACE = getattr(res, "instructions_and_trace", None)

    out = np.empty((B, N, D), dtype=np.float32)
    for b in range(B):
        out[b] = res.results[2 * b]["out"] + res.results[2 * b + 1]["out"]
    return out
